# revision 27
# baseline (speedup 1.0000x reference)
"""AI4DEM contact-force stencil on 8 TRN2 NeuronCores.

Math: for each neighbor offset o=(oy,ox) in the 5x5 window,
  dx = x - shift(x, o), dy likewise, dist = sqrt(dx^2+dy^2)
  Fx_o = where(dist < 2d, kn*(dist-2d)/max(eps,dist) * dx, 0)
       = -kn * relu(2d/dist - 1) * dx    (clamped at dist<eps)
  fx = mask * sum_o Fx_o

Mirror symmetry: Fx_{-o}(p) = -Fx_o(p+o), so only the 12 half-offsets
H = {oy>0} u {oy=0, ox>0} are computed:
  fx(p) = sum_{o in H} px_o(p) - px_o(p+o),   px_o = w_o * dx_o
The shifted term is accumulated by TensorE matmuls with +/-1 shift
matrices into PSUM (partition shifts), free-dim reads handle ox.
The 2 rows past each 128-row tile (and past the core's 256-row block)
come from one packed 48-partition "seam" chain over host-gathered rows.

1/dist is one ACT op Abs_reciprocal_sqrt(sq + 4.1e-6); grids are
pre-scaled by 64 on the host (exact in f32) so fp16 sq stays normal
for all dist >= ~eps; all scale factors fold into existing constants.
dx/dy are f32 subs (cancellation) cast to fp16 on write; the rest of
the chain runs fp16 in DVE 2x/4x perf modes.
"""

import numpy as np

NY = NX = 2048
NCORES = 8
ROWS = NY // NCORES          # 256 rows per core
TILE = 128
NT = ROWS // TILE            # 2 row-tiles per core
W = NX + 4                   # px width (2-col halo)
WS = NX + 8                  # slab width (4-col halo)
EPS = 1e-4
SCALE = 64.0

# half-offsets, oy ascending so oy=0 pairs start right after the first loads
HOFF = [(0, 1), (0, 2)] + [(oy, ox) for oy in (1, 2) for ox in (-2, -1, 0, 1, 2)]
NH = len(HOFF)               # 12

_cache = {}
LAST_RESULTS = None


def _build(d_val: float, kn_val: float):
    import concourse.tile as tile
    from concourse import bacc, mybir

    f32 = mybir.dt.float32
    f16 = mybir.dt.float16
    AF = mybir.ActivationFunctionType
    OP = mybir.AluOpType

    nc = bacc.Bacc("TRN2", target_bir_lowering=False, debug=False,
                   enable_asserts=False, num_devices=NCORES)
    xs_ext = nc.declare_dram_parameter("xs", [ROWS + 4, WS], f32, isOutput=False)
    ys_ext = nc.declare_dram_parameter("ys", [ROWS + 4, WS], f32, isOutput=False)
    ms_ext = nc.declare_dram_parameter("ms", [ROWS, NX], f32, isOutput=False)
    # lhs blocks: [I, -S0, -S1, -S2] where Sk shifts partitions by k
    lhs_ext = nc.declare_dram_parameter("lhs", [128, 6 * 128], f16, isOutput=False)
    # boundary lhsT blocks per (ox, t): [48, 10*128]
    lhsb_ext = nc.declare_dram_parameter("lhsb", [48, 10 * 128], f16,
                                         isOutput=False)
    sax_ext = nc.declare_dram_parameter("sax", [4 * NH, W], f32, isOutput=False)
    sbx_ext = nc.declare_dram_parameter("sbx", [4 * NH, W], f32, isOutput=False)
    say_ext = nc.declare_dram_parameter("say", [4 * NH, W], f32, isOutput=False)
    sby_ext = nc.declare_dram_parameter("sby", [4 * NH, W], f32, isOutput=False)
    out_ext = nc.declare_dram_parameter("out", [2, ROWS, NX], f32, isOutput=True)

    SP = 4 * NH              # 48 seam partitions

    with tile.TileContext(nc) as tc:
        with tc.tile_pool(name="const", bufs=1) as cpool, \
             tc.tile_pool(name="xin", bufs=1) as xpool, \
             tc.tile_pool(name="stg", bufs=2) as spool, \
             tc.tile_pool(name="tmp", bufs=3) as tpool, \
             tc.tile_pool(name="rp", bufs=4) as rpool, \
             tc.tile_pool(name="pxy", bufs=3) as ppool, \
             tc.tile_pool(name="outp", bufs=1) as opool, \
             tc.tile_pool(name="acc", bufs=1, space="PSUM") as psum_pool:

            lhs_t = cpool.tile([128, 6 * 128], f16)
            nc.sync.dma_start(lhs_t[:], lhs_ext[:])
            lhsb_t = cpool.tile([48, 10 * 128], f16)
            nc.sync.dma_start(lhsb_t[:], lhsb_ext[:])
            floor_b = cpool.tile([128, 1], f32)
            nc.vector.memset(floor_b[:], 4.1e-6)
            neg1 = cpool.tile([128, 1], f32)
            nc.vector.memset(neg1[:], -1.0)

            def chain(dxa, dxb, dya, dyb, P, width, px_tag):
                """dx = dxa - dxb etc; returns (px, py) fp16 [P, width]."""
                dx = tpool.tile([P, width], f16, tag="dx")
                dy = tpool.tile([P, width], f16, tag="dy")
                nc.vector.tensor_sub(dx[:], dxa, dxb)
                nc.vector.tensor_sub(dy[:], dya, dyb)
                sqx = tpool.tile([P, width], f16, tag="sqx")
                nc.scalar.activation(sqx[:], dx[:], AF.Square)
                sqy = tpool.tile([P, width], f16, tag="sqy")
                nc.scalar.activation(sqy[:], dy[:], AF.Square)
                nc.vector.tensor_add(sqx[:], sqx[:], sqy[:])
                rec = rpool.tile([P, width], f16, tag="rec")
                nc.scalar.activation(rec[:], sqx[:], AF.Abs_reciprocal_sqrt,
                                     bias=floor_b[:P])
                nc.scalar.activation(rec[:], rec[:], AF.Relu,
                                     bias=neg1[:P], scale=2.0 * SCALE * d_val)
                px = ppool.tile([P, width], f16, tag=f"px{px_tag}")
                py = ppool.tile([P, width], f16, tag=f"py{px_tag}")
                nc.vector.tensor_mul(px[:], rec[:], dx[:])
                nc.vector.tensor_mul(py[:], rec[:], dy[:])
                return px, py

            # ---- seam chain: rows {128,129,256,257}+R0 for all 12 offsets
            sax = spool.tile([SP, W], f32, tag="sa")
            nc.sync.dma_start(sax[:], sax_ext[:])
            sbx = spool.tile([SP, W], f32, tag="sb")
            nc.sync.dma_start(sbx[:], sbx_ext[:])
            say = spool.tile([SP, W], f32, tag="sa")
            nc.sync.dma_start(say[:], say_ext[:])
            sby = spool.tile([SP, W], f32, tag="sb")
            nc.sync.dma_start(sby[:], sby_ext[:])
            dxm = tpool.tile([SP, W], f16, tag="dx")
            dym = tpool.tile([SP, W], f16, tag="dy")
            nc.vector.tensor_sub(dxm[:], sax[:], sbx[:])
            nc.vector.tensor_sub(dym[:], say[:], sby[:])
            sqxm = tpool.tile([SP, W], f16, tag="sqx")
            nc.scalar.activation(sqxm[:], dxm[:], AF.Square)
            sqym = tpool.tile([SP, W], f16, tag="sqy")
            nc.scalar.activation(sqym[:], dym[:], AF.Square)
            nc.vector.tensor_add(sqxm[:], sqxm[:], sqym[:])
            recm_ = rpool.tile([SP, W], f16, tag="rec")
            nc.scalar.activation(recm_[:], sqxm[:], AF.Abs_reciprocal_sqrt,
                                 bias=floor_b[:SP])
            nc.scalar.activation(recm_[:], recm_[:], AF.Relu,
                                 bias=neg1[:SP], scale=2.0 * SCALE * d_val)
            pxm = cpool.tile([SP, W], f16, tag="pxm")
            pym = cpool.tile([SP, W], f16, tag="pym")
            nc.vector.tensor_mul(pxm[:], recm_[:], dxm[:])
            nc.vector.tensor_mul(pym[:], recm_[:], dym[:])

            for t in range(NT):
                t0 = t * TILE
                X = {}
                Y = {}
                for s in (0, -1, -2):
                    xt = xpool.tile([TILE, WS], f32, tag=f"xs{s}")
                    nc.sync.dma_start(
                        xt[:], xs_ext[t0 + s + 2: t0 + s + 2 + TILE, :])
                    X[s] = xt
                    yt = xpool.tile([TILE, WS], f32, tag=f"ys{s}")
                    nc.sync.dma_start(
                        yt[:], ys_ext[t0 + s + 2: t0 + s + 2 + TILE, :])
                    Y[s] = yt
                mask_t = xpool.tile([TILE, NX], f32, tag="mask")
                nc.sync.dma_start(mask_t[:], ms_ext[t0: t0 + TILE, :])

                fx_ps = psum_pool.tile([TILE, NX], f32, tag="fx")
                fy_ps = psum_pool.tile([TILE, NX], f32, tag="fy")

                for oi, (oy, ox) in enumerate(HOFF):
                    first = oi == 0
                    last = oi == NH - 1
                    # dx_o[p,u] = x[t0+p, u-2] - x[t0+p-oy, u-2-ox], u in [0,W)
                    px, py = chain(
                        X[0][:, 2: 2 + W], X[-oy][:, 2 - ox: 2 - ox + W],
                        Y[0][:, 2: 2 + W], Y[-oy][:, 2 - ox: 2 - ox + W],
                        TILE, W, "t")
                    if oi == 1:
                        # seam boundary mms (need only pxm/pym; must follow
                        # the bank-clearing start mms of oi==0)
                        for bi, bx in enumerate((-2, -1, 0, 1, 2)):
                            for pm, acc in ((pxm, fx_ps), (pym, fy_ps)):
                                for k in range(NX // 512):
                                    nc.tensor.matmul(
                                        acc[:, 512 * k: 512 * k + 512],
                                        lhsb_t[:, 128 * (2 * bi + t):
                                               128 * (2 * bi + t) + 128],
                                        pm[:, 2 + 512 * k + bx:
                                           2 + 512 * k + bx + 512],
                                        start=False, stop=False)
                    for ps, pm, acc in ((px, pxm, fx_ps), (py, pym, fy_ps)):
                        if ox == 0:
                            # merged lhsT block (I - S(oy)): one mm per chunk
                            for k in range(NX // 512):
                                nc.tensor.matmul(
                                    acc[:, 512 * k: 512 * k + 512],
                                    lhs_t[:, 128 * (3 + oy): 128 * (4 + oy)],
                                    ps[:, 2 + 512 * k: 2 + 512 * k + 512],
                                    start=first, stop=False)
                        else:
                            for k in range(NX // 512):
                                nc.tensor.matmul(
                                    acc[:, 512 * k: 512 * k + 512],
                                    lhs_t[:, 0:128],
                                    ps[:, 2 + 512 * k: 2 + 512 * k + 512],
                                    start=first, stop=False)
                            for k in range(NX // 512):
                                nc.tensor.matmul(
                                    acc[:, 512 * k: 512 * k + 512],
                                    lhs_t[:, 128 * (1 + oy): 128 * (2 + oy)],
                                    ps[:, 2 + 512 * k + ox:
                                       2 + 512 * k + ox + 512],
                                    start=False, stop=last)

                fx_sb = opool.tile([TILE, NX], f32, tag="fxsb")
                fy_sb = opool.tile([TILE, NX], f32, tag="fysb")
                nc.vector.scalar_tensor_tensor(
                    fx_sb[:], fx_ps[:], -float(kn_val) / SCALE, mask_t[:],
                    OP.mult, OP.mult)
                nc.vector.scalar_tensor_tensor(
                    fy_sb[:], fy_ps[:], -float(kn_val) / SCALE, mask_t[:],
                    OP.mult, OP.mult)
                nc.sync.dma_start(out_ext[0, t0: t0 + TILE, :], fx_sb[:])
                nc.sync.dma_start(out_ext[1, t0: t0 + TILE, :], fy_sb[:])

    nc.compile()
    return nc


def _host_inputs(gx, gy, ms):
    """Per-core input dict list. gx/gy already scaled by 64 (f32-exact)."""
    eye = np.eye(128, dtype=np.float16)
    lhs = np.zeros((128, 6 * 128), dtype=np.float16)
    lhs[:, 0:128] = eye
    for oy in (0, 1, 2):
        blk = np.zeros((128, 128), dtype=np.float16)
        for m in range(128 - oy):
            blk[m + oy, m] = -1.0
        lhs[:, 128 * (1 + oy): 128 * (2 + oy)] = blk
        if oy > 0:
            lhs[:, 128 * (3 + oy): 128 * (4 + oy)] = eye + blk
    lhsb = np.zeros((48, 10 * 128), dtype=np.float16)
    for bi, bx in enumerate((-2, -1, 0, 1, 2)):
        for t in (0, 1):
            blk = np.zeros((48, 128), dtype=np.float16)
            for oy in (1, 2):
                oi = HOFF.index((oy, bx))
                for m in range(128 - oy, 128):
                    blk[4 * oi + 2 * t + (m + oy - 128), m] = -1.0
            lhsb[:, 128 * (2 * bi + t): 128 * (2 * bi + t) + 128] = blk

    cols = np.arange(-2, NX + 2) % NX         # width W, col u -> u-2
    colss = np.arange(-4, NX + 4) % NX        # width WS, col v -> v-4
    in_maps = []
    for i in range(NCORES):
        r0 = i * ROWS
        rows = np.arange(r0 - 2, r0 + ROWS + 2) % NY
        # seam rows per (oi, j): j in {0,1}: r0+128+j ; j in {2,3}: r0+256+(j-2)
        sa_rows = np.empty(4 * NH, dtype=np.int64)
        sb_rows = np.empty(4 * NH, dtype=np.int64)
        sb_cols = np.empty((4 * NH, W), dtype=np.int64)
        for oi, (oy, ox) in enumerate(HOFF):
            for j in range(4):
                row = r0 + 128 + j if j < 2 else r0 + 256 + (j - 2)
                sa_rows[4 * oi + j] = row % NY
                sb_rows[4 * oi + j] = (row - oy) % NY
                sb_cols[4 * oi + j] = (cols - ox) % NX
        in_maps.append({
            "xs": np.ascontiguousarray(gx[np.ix_(rows, colss)]),
            "ys": np.ascontiguousarray(gy[np.ix_(rows, colss)]),
            "ms": np.ascontiguousarray(ms[r0: r0 + ROWS, :]),
            "lhs": lhs,
            "lhsb": lhsb,
            "sax": np.ascontiguousarray(gx[sa_rows][:, cols]),
            "sbx": np.ascontiguousarray(gx[sb_rows[:, None], sb_cols]),
            "say": np.ascontiguousarray(gy[sa_rows][:, cols]),
            "sby": np.ascontiguousarray(gy[sb_rows[:, None], sb_cols]),
        })
    return in_maps


def _install_profile_hook():
    """The image's antenv lacks axon_hooks; recreate it so trace=True can
    drive NTFF profiling through libaxon_pjrt (local-only, no upload)."""
    import sys
    import types

    if "antenv.axon_hooks" not in sys.modules:
        mod = types.ModuleType("antenv.axon_hooks")
        holder = {}
        mod.set_axon_ntff_profile_hook = lambda h: holder.__setitem__("h", h)
        mod.get_axon_ntff_profile_hook = lambda: holder.get("h")
        sys.modules["antenv.axon_hooks"] = mod
        try:
            import antenv
            antenv.axon_hooks = mod
        except ImportError:
            pass
        if "/root/.axon_site" not in sys.path:
            sys.path.insert(0, "/root/.axon_site")
        from trn_agent_boot.trn_boot import _ntff_profile_via_ctypes
        h = _ntff_profile_via_ctypes("/opt/axon/libaxon_pjrt.so")
        if h is not None:
            mod.set_axon_ntff_profile_hook(h)
    from concourse import bass_utils as bu
    bu.upload_artifacts = lambda tmpdir: ""


def kernel(grid_x, grid_y, mask, d=1, kn=100, **_unused):
    global LAST_RESULTS
    from concourse.bass_utils import run_bass_kernel_spmd
    from concourse.bass_utils import checkenv

    if checkenv("KERNEL_TRACE"):
        _install_profile_hook()

    d_val = float(np.asarray(d))
    kn_val = float(np.asarray(kn))
    key = (d_val, kn_val)
    if key not in _cache:
        _cache[key] = _build(d_val, kn_val)
    nc = _cache[key]

    gx = np.asarray(grid_x, dtype=np.float32)[0, 0] * np.float32(SCALE)
    gy = np.asarray(grid_y, dtype=np.float32)[0, 0] * np.float32(SCALE)
    ms = np.asarray(mask, dtype=np.float32)[0, 0]
    in_maps = _host_inputs(gx, gy, ms)

    res = run_bass_kernel_spmd(nc, in_maps, core_ids=list(range(NCORES)),
                               trace=bool(checkenv("KERNEL_TRACE")))
    LAST_RESULTS = res

    fx = np.concatenate([res.results[i]["out"][0] for i in range(NCORES)], axis=0)
    fy = np.concatenate([res.results[i]["out"][1] for i in range(NCORES)], axis=0)
    fx = fx.reshape(1, 1, NY, NX)
    fy = fy.reshape(1, 1, NY, NX)
    return fx, fy


# revision 28
# speedup vs baseline: 1.1863x; 1.1863x over previous
"""AI4DEM contact-force stencil on 8 TRN2 NeuronCores.

Math: for each neighbor offset o=(oy,ox) in the 5x5 window,
  dx = x - shift(x, o), dy likewise, dist = sqrt(dx^2+dy^2)
  Fx_o = where(dist < 2d, kn*(dist-2d)/max(eps,dist) * dx, 0)
       = -kn * relu(2d/dist - 1) * dx    (clamped at dist<eps)
  fx = mask * sum_o Fx_o

Mirror symmetry: Fx_{-o}(p) = -Fx_o(p+o), so only the 12 half-offsets
H = {oy>0} u {oy=0, ox>0} are computed:
  fx(p) = sum_{o in H} px_o(p) - px_o(p+o),   px_o = w_o * dx_o
The shifted term is accumulated by TensorE matmuls with +/-1 shift
matrices into PSUM (partition shifts), free-dim reads handle ox.
The 2 rows past each 128-row tile (and past the core's 256-row block)
come from one packed 48-partition "seam" chain over host-gathered rows.

1/dist is one ACT op Abs_reciprocal_sqrt(sq + 4.1e-6); grids are
pre-scaled by 64 on the host (exact in f32) so fp16 sq stays normal
for all dist >= ~eps; all scale factors fold into existing constants.
dx/dy are f32 subs (cancellation) cast to fp16 on write; the rest of
the chain runs fp16 in DVE 2x/4x perf modes.
"""

import numpy as np

NY = NX = 2048
NCORES = 8
ROWS = NY // NCORES          # 256 rows per core
TILE = 128
NT = ROWS // TILE            # 2 row-tiles per core
W = NX + 4                   # px width (2-col halo)
WS = NX + 8                  # slab width (4-col halo)
EPS = 1e-4
SCALE = 64.0

# half-offsets, oy ascending so oy=0 pairs start right after the first loads
HOFF = [(0, 1), (0, 2)] + [(oy, ox) for oy in (1, 2) for ox in (-2, -1, 0, 1, 2)]
NH = len(HOFF)               # 12

_cache = {}
LAST_RESULTS = None


def _build(d_val: float, kn_val: float):
    import concourse.tile as tile
    from concourse import bacc, mybir

    f32 = mybir.dt.float32
    f16 = mybir.dt.float16
    AF = mybir.ActivationFunctionType
    OP = mybir.AluOpType

    nc = bacc.Bacc("TRN2", target_bir_lowering=False, debug=False,
                   enable_asserts=False, num_devices=NCORES)
    xs_ext = nc.declare_dram_parameter("xs", [ROWS + 4, WS], f32, isOutput=False)
    ys_ext = nc.declare_dram_parameter("ys", [ROWS + 4, WS], f32, isOutput=False)
    ms_ext = nc.declare_dram_parameter("ms", [ROWS, NX], f32, isOutput=False)
    # lhs blocks: [I, -S0, -S1, -S2] where Sk shifts partitions by k
    lhs_ext = nc.declare_dram_parameter("lhs", [128, 6 * 128], f16, isOutput=False)
    # boundary lhsT blocks per (ox, t): [48, 10*128]
    lhsb_ext = nc.declare_dram_parameter("lhsb", [48, 10 * 128], f16,
                                         isOutput=False)
    sax_ext = nc.declare_dram_parameter("sax", [4 * NH, W], f32, isOutput=False)
    sbx_ext = nc.declare_dram_parameter("sbx", [4 * NH, W], f32, isOutput=False)
    say_ext = nc.declare_dram_parameter("say", [4 * NH, W], f32, isOutput=False)
    sby_ext = nc.declare_dram_parameter("sby", [4 * NH, W], f32, isOutput=False)
    out_ext = nc.declare_dram_parameter("out", [2, ROWS, NX], f32, isOutput=True)

    SP = 4 * NH              # 48 seam partitions

    with tile.TileContext(nc) as tc:
        with tc.tile_pool(name="const", bufs=1) as cpool, \
             tc.tile_pool(name="xin", bufs=1) as xpool, \
             tc.tile_pool(name="stg", bufs=2) as spool, \
             tc.tile_pool(name="tmp", bufs=3) as tpool, \
             tc.tile_pool(name="pxy", bufs=3) as ppool, \
             tc.tile_pool(name="outp", bufs=1) as opool, \
             tc.tile_pool(name="acc", bufs=1, space="PSUM") as psum_pool:

            lhs_t = cpool.tile([128, 6 * 128], f16)
            nc.sync.dma_start(lhs_t[:], lhs_ext[:])
            lhsb_t = cpool.tile([48, 10 * 128], f16)
            nc.sync.dma_start(lhsb_t[:], lhsb_ext[:])
            floor_b = cpool.tile([128, 1], f32)
            nc.vector.memset(floor_b[:], 4.1e-6)
            neg1 = cpool.tile([128, 1], f32)
            nc.vector.memset(neg1[:], -1.0)

            def chain(dxa, dxb, dya, dyb, P, width, px_tag):
                """dx = dxa - dxb etc; returns (px, py) fp16 [P, width]."""
                dx = tpool.tile([P, width], f16, tag="dx")
                dy = tpool.tile([P, width], f16, tag="dy")
                nc.vector.tensor_sub(dx[:], dxa, dxb)
                nc.vector.tensor_sub(dy[:], dya, dyb)
                sqx = tpool.tile([P, width], f16, tag="sqx")
                nc.scalar.activation(sqx[:], dx[:], AF.Square)
                sqy = tpool.tile([P, width], f16, tag="sqy")
                nc.scalar.activation(sqy[:], dy[:], AF.Square)
                nc.vector.tensor_add(sqx[:], sqx[:], sqy[:])
                rec = tpool.tile([P, width], f16, tag="rec")
                nc.scalar.activation(rec[:], sqx[:], AF.Abs_reciprocal_sqrt,
                                     bias=floor_b[:P])
                nc.scalar.activation(rec[:], rec[:], AF.Relu,
                                     bias=neg1[:P], scale=2.0 * SCALE * d_val)
                px = ppool.tile([P, width], f16, tag=f"px{px_tag}")
                py = ppool.tile([P, width], f16, tag=f"py{px_tag}")
                nc.vector.tensor_mul(px[:], rec[:], dx[:])
                nc.vector.tensor_mul(py[:], rec[:], dy[:])
                return px, py

            # ---- seam chain: rows {128,129,256,257}+R0 for all 12 offsets
            sax = spool.tile([SP, W], f32, tag="sa")
            nc.sync.dma_start(sax[:], sax_ext[:])
            sbx = spool.tile([SP, W], f32, tag="sb")
            nc.sync.dma_start(sbx[:], sbx_ext[:])
            say = spool.tile([SP, W], f32, tag="sa")
            nc.sync.dma_start(say[:], say_ext[:])
            sby = spool.tile([SP, W], f32, tag="sb")
            nc.sync.dma_start(sby[:], sby_ext[:])
            dxm = tpool.tile([SP, W], f16, tag="dx")
            dym = tpool.tile([SP, W], f16, tag="dy")
            nc.vector.tensor_sub(dxm[:], sax[:], sbx[:])
            nc.vector.tensor_sub(dym[:], say[:], sby[:])
            sqxm = tpool.tile([SP, W], f16, tag="sqx")
            nc.scalar.activation(sqxm[:], dxm[:], AF.Square)
            sqym = tpool.tile([SP, W], f16, tag="sqy")
            nc.scalar.activation(sqym[:], dym[:], AF.Square)
            nc.vector.tensor_add(sqxm[:], sqxm[:], sqym[:])
            recm_ = tpool.tile([SP, W], f16, tag="rec")
            nc.scalar.activation(recm_[:], sqxm[:], AF.Abs_reciprocal_sqrt,
                                 bias=floor_b[:SP])
            nc.scalar.activation(recm_[:], recm_[:], AF.Relu,
                                 bias=neg1[:SP], scale=2.0 * SCALE * d_val)
            pxm = cpool.tile([SP, W], f16, tag="pxm")
            pym = cpool.tile([SP, W], f16, tag="pym")
            nc.vector.tensor_mul(pxm[:], recm_[:], dxm[:])
            nc.vector.tensor_mul(pym[:], recm_[:], dym[:])

            for t in range(NT):
                t0 = t * TILE
                X = {}
                Y = {}
                for s in (0, -1, -2):
                    xt = xpool.tile([TILE, WS], f32, tag=f"xs{s}")
                    nc.sync.dma_start(
                        xt[:], xs_ext[t0 + s + 2: t0 + s + 2 + TILE, :])
                    X[s] = xt
                    yt = xpool.tile([TILE, WS], f32, tag=f"ys{s}")
                    nc.sync.dma_start(
                        yt[:], ys_ext[t0 + s + 2: t0 + s + 2 + TILE, :])
                    Y[s] = yt
                mask_t = xpool.tile([TILE, NX], f32, tag="mask")
                nc.sync.dma_start(mask_t[:], ms_ext[t0: t0 + TILE, :])

                fx_ps = psum_pool.tile([TILE, NX], f32, tag="fx")
                fy_ps = psum_pool.tile([TILE, NX], f32, tag="fy")

                for oi, (oy, ox) in enumerate(HOFF):
                    first = oi == 0
                    last = oi == NH - 1
                    # dx_o[p,u] = x[t0+p, u-2] - x[t0+p-oy, u-2-ox], u in [0,W)
                    px, py = chain(
                        X[0][:, 2: 2 + W], X[-oy][:, 2 - ox: 2 - ox + W],
                        Y[0][:, 2: 2 + W], Y[-oy][:, 2 - ox: 2 - ox + W],
                        TILE, W, "t")
                    if oi == 1:
                        # seam boundary mms (need only pxm/pym; must follow
                        # the bank-clearing start mms of oi==0)
                        for bi, bx in enumerate((-2, -1, 0, 1, 2)):
                            for pm, acc in ((pxm, fx_ps), (pym, fy_ps)):
                                for k in range(NX // 512):
                                    nc.tensor.matmul(
                                        acc[:, 512 * k: 512 * k + 512],
                                        lhsb_t[:, 128 * (2 * bi + t):
                                               128 * (2 * bi + t) + 128],
                                        pm[:, 2 + 512 * k + bx:
                                           2 + 512 * k + bx + 512],
                                        start=False, stop=False)
                    for ps, pm, acc in ((px, pxm, fx_ps), (py, pym, fy_ps)):
                        if ox == 0:
                            # merged lhsT block (I - S(oy)): one mm per chunk
                            for k in range(NX // 512):
                                nc.tensor.matmul(
                                    acc[:, 512 * k: 512 * k + 512],
                                    lhs_t[:, 128 * (3 + oy): 128 * (4 + oy)],
                                    ps[:, 2 + 512 * k: 2 + 512 * k + 512],
                                    start=first, stop=False)
                        else:
                            for k in range(NX // 512):
                                nc.tensor.matmul(
                                    acc[:, 512 * k: 512 * k + 512],
                                    lhs_t[:, 0:128],
                                    ps[:, 2 + 512 * k: 2 + 512 * k + 512],
                                    start=first, stop=False)
                            for k in range(NX // 512):
                                nc.tensor.matmul(
                                    acc[:, 512 * k: 512 * k + 512],
                                    lhs_t[:, 128 * (1 + oy): 128 * (2 + oy)],
                                    ps[:, 2 + 512 * k + ox:
                                       2 + 512 * k + ox + 512],
                                    start=False, stop=last)

                fx_sb = opool.tile([TILE, NX], f32, tag="fxsb")
                fy_sb = opool.tile([TILE, NX], f32, tag="fysb")
                nc.vector.scalar_tensor_tensor(
                    fx_sb[:], fx_ps[:], -float(kn_val) / SCALE, mask_t[:],
                    OP.mult, OP.mult)
                nc.vector.scalar_tensor_tensor(
                    fy_sb[:], fy_ps[:], -float(kn_val) / SCALE, mask_t[:],
                    OP.mult, OP.mult)
                nc.sync.dma_start(out_ext[0, t0: t0 + TILE, :], fx_sb[:])
                nc.sync.dma_start(out_ext[1, t0: t0 + TILE, :], fy_sb[:])

    nc.compile()
    return nc


def _host_inputs(gx, gy, ms):
    """Per-core input dict list. gx/gy already scaled by 64 (f32-exact)."""
    eye = np.eye(128, dtype=np.float16)
    lhs = np.zeros((128, 6 * 128), dtype=np.float16)
    lhs[:, 0:128] = eye
    for oy in (0, 1, 2):
        blk = np.zeros((128, 128), dtype=np.float16)
        for m in range(128 - oy):
            blk[m + oy, m] = -1.0
        lhs[:, 128 * (1 + oy): 128 * (2 + oy)] = blk
        if oy > 0:
            lhs[:, 128 * (3 + oy): 128 * (4 + oy)] = eye + blk
    lhsb = np.zeros((48, 10 * 128), dtype=np.float16)
    for bi, bx in enumerate((-2, -1, 0, 1, 2)):
        for t in (0, 1):
            blk = np.zeros((48, 128), dtype=np.float16)
            for oy in (1, 2):
                oi = HOFF.index((oy, bx))
                for m in range(128 - oy, 128):
                    blk[4 * oi + 2 * t + (m + oy - 128), m] = -1.0
            lhsb[:, 128 * (2 * bi + t): 128 * (2 * bi + t) + 128] = blk

    cols = np.arange(-2, NX + 2) % NX         # width W, col u -> u-2
    colss = np.arange(-4, NX + 4) % NX        # width WS, col v -> v-4
    in_maps = []
    for i in range(NCORES):
        r0 = i * ROWS
        rows = np.arange(r0 - 2, r0 + ROWS + 2) % NY
        # seam rows per (oi, j): j in {0,1}: r0+128+j ; j in {2,3}: r0+256+(j-2)
        sa_rows = np.empty(4 * NH, dtype=np.int64)
        sb_rows = np.empty(4 * NH, dtype=np.int64)
        sb_cols = np.empty((4 * NH, W), dtype=np.int64)
        for oi, (oy, ox) in enumerate(HOFF):
            for j in range(4):
                row = r0 + 128 + j if j < 2 else r0 + 256 + (j - 2)
                sa_rows[4 * oi + j] = row % NY
                sb_rows[4 * oi + j] = (row - oy) % NY
                sb_cols[4 * oi + j] = (cols - ox) % NX
        in_maps.append({
            "xs": np.ascontiguousarray(gx[np.ix_(rows, colss)]),
            "ys": np.ascontiguousarray(gy[np.ix_(rows, colss)]),
            "ms": np.ascontiguousarray(ms[r0: r0 + ROWS, :]),
            "lhs": lhs,
            "lhsb": lhsb,
            "sax": np.ascontiguousarray(gx[sa_rows][:, cols]),
            "sbx": np.ascontiguousarray(gx[sb_rows[:, None], sb_cols]),
            "say": np.ascontiguousarray(gy[sa_rows][:, cols]),
            "sby": np.ascontiguousarray(gy[sb_rows[:, None], sb_cols]),
        })
    return in_maps


def _install_profile_hook():
    """The image's antenv lacks axon_hooks; recreate it so trace=True can
    drive NTFF profiling through libaxon_pjrt (local-only, no upload)."""
    import sys
    import types

    if "antenv.axon_hooks" not in sys.modules:
        mod = types.ModuleType("antenv.axon_hooks")
        holder = {}
        mod.set_axon_ntff_profile_hook = lambda h: holder.__setitem__("h", h)
        mod.get_axon_ntff_profile_hook = lambda: holder.get("h")
        sys.modules["antenv.axon_hooks"] = mod
        try:
            import antenv
            antenv.axon_hooks = mod
        except ImportError:
            pass
        if "/root/.axon_site" not in sys.path:
            sys.path.insert(0, "/root/.axon_site")
        from trn_agent_boot.trn_boot import _ntff_profile_via_ctypes
        h = _ntff_profile_via_ctypes("/opt/axon/libaxon_pjrt.so")
        if h is not None:
            mod.set_axon_ntff_profile_hook(h)
    from concourse import bass_utils as bu
    bu.upload_artifacts = lambda tmpdir: ""


def kernel(grid_x, grid_y, mask, d=1, kn=100, **_unused):
    global LAST_RESULTS
    from concourse.bass_utils import run_bass_kernel_spmd
    from concourse.bass_utils import checkenv

    if checkenv("KERNEL_TRACE"):
        _install_profile_hook()

    d_val = float(np.asarray(d))
    kn_val = float(np.asarray(kn))
    key = (d_val, kn_val)
    if key not in _cache:
        _cache[key] = _build(d_val, kn_val)
    nc = _cache[key]

    gx = np.asarray(grid_x, dtype=np.float32)[0, 0] * np.float32(SCALE)
    gy = np.asarray(grid_y, dtype=np.float32)[0, 0] * np.float32(SCALE)
    ms = np.asarray(mask, dtype=np.float32)[0, 0]
    in_maps = _host_inputs(gx, gy, ms)

    res = run_bass_kernel_spmd(nc, in_maps, core_ids=list(range(NCORES)),
                               trace=bool(checkenv("KERNEL_TRACE")))
    LAST_RESULTS = res

    fx = np.concatenate([res.results[i]["out"][0] for i in range(NCORES)], axis=0)
    fy = np.concatenate([res.results[i]["out"][1] for i in range(NCORES)], axis=0)
    fx = fx.reshape(1, 1, NY, NX)
    fy = fy.reshape(1, 1, NY, NX)
    return fx, fy


# revision 29
# speedup vs baseline: 1.1911x; 1.0041x over previous
"""AI4DEM contact-force stencil on 8 TRN2 NeuronCores.

Math: for each neighbor offset o=(oy,ox) in the 5x5 window,
  dx = x - shift(x, o), dy likewise, dist = sqrt(dx^2+dy^2)
  Fx_o = where(dist < 2d, kn*(dist-2d)/max(eps,dist) * dx, 0)
       = -kn * relu(2d/dist - 1) * dx    (clamped at dist<eps)
  fx = mask * sum_o Fx_o

Mirror symmetry: Fx_{-o}(p) = -Fx_o(p+o), so only the 12 half-offsets
H = {oy>0} u {oy=0, ox>0} are computed:
  fx(p) = sum_{o in H} px_o(p) - px_o(p+o),   px_o = w_o * dx_o
The shifted term is accumulated by TensorE matmuls with +/-1 shift
matrices into PSUM (partition shifts), free-dim reads handle ox.
The 2 rows past each 128-row tile (and past the core's 256-row block)
come from one packed 48-partition "seam" chain over host-gathered rows.

1/dist is one ACT op Abs_reciprocal_sqrt(sq + 4.1e-6); grids are
pre-scaled by 64 on the host (exact in f32) so fp16 sq stays normal
for all dist >= ~eps; all scale factors fold into existing constants.
dx/dy are f32 subs (cancellation) cast to fp16 on write; the rest of
the chain runs fp16 in DVE 2x/4x perf modes.
"""

import numpy as np

NY = NX = 2048
NCORES = 8
ROWS = NY // NCORES          # 256 rows per core
TILE = 128
NT = ROWS // TILE            # 2 row-tiles per core
W = NX + 4                   # px width (2-col halo)
WS = NX + 8                  # slab width (4-col halo)
EPS = 1e-4
SCALE = 64.0

# half-offsets, oy ascending so oy=0 pairs start right after the first loads
HOFF = [(0, 1), (0, 2)] + [(oy, ox) for oy in (1, 2) for ox in (-2, -1, 0, 1, 2)]
NH = len(HOFF)               # 12

_cache = {}
LAST_RESULTS = None


def _build(d_val: float, kn_val: float):
    import concourse.tile as tile
    from concourse import bacc, mybir

    f32 = mybir.dt.float32
    f16 = mybir.dt.float16
    AF = mybir.ActivationFunctionType
    OP = mybir.AluOpType

    nc = bacc.Bacc("TRN2", target_bir_lowering=False, debug=False,
                   enable_asserts=False, num_devices=NCORES)
    xs_ext = nc.declare_dram_parameter("xs", [ROWS + 4, WS], f32, isOutput=False)
    ys_ext = nc.declare_dram_parameter("ys", [ROWS + 4, WS], f32, isOutput=False)
    ms_ext = nc.declare_dram_parameter("ms", [ROWS, NX], f32, isOutput=False)
    # lhs blocks: [I, -S0, -S1, -S2] where Sk shifts partitions by k
    lhs_ext = nc.declare_dram_parameter("lhs", [128, 6 * 128], f16, isOutput=False)
    # boundary lhsT blocks per (ox, t): [48, 10*128]
    lhsb_ext = nc.declare_dram_parameter("lhsb", [48, 10 * 128], f16,
                                         isOutput=False)
    sax_ext = nc.declare_dram_parameter("sax", [4 * NH, W], f32, isOutput=False)
    sbx_ext = nc.declare_dram_parameter("sbx", [4 * NH, W], f32, isOutput=False)
    say_ext = nc.declare_dram_parameter("say", [4 * NH, W], f32, isOutput=False)
    sby_ext = nc.declare_dram_parameter("sby", [4 * NH, W], f32, isOutput=False)
    out_ext = nc.declare_dram_parameter("out", [2, ROWS, NX], f32, isOutput=True)

    SP = 4 * NH              # 48 seam partitions

    with tile.TileContext(nc) as tc:
        with tc.tile_pool(name="const", bufs=1) as cpool, \
             tc.tile_pool(name="xin", bufs=1) as xpool, \
             tc.tile_pool(name="stg", bufs=2) as spool, \
             tc.tile_pool(name="tmp", bufs=3) as tpool, \
             tc.tile_pool(name="pxy", bufs=3) as ppool, \
             tc.tile_pool(name="outp", bufs=1) as opool, \
             tc.tile_pool(name="acc", bufs=1, space="PSUM") as psum_pool:

            lhs_t = cpool.tile([128, 6 * 128], f16)
            nc.sync.dma_start(lhs_t[:], lhs_ext[:])
            lhsb_t = cpool.tile([48, 10 * 128], f16)
            nc.sync.dma_start(lhsb_t[:], lhsb_ext[:])
            floor_b = cpool.tile([128, 1], f32)
            nc.vector.memset(floor_b[:], 4.1e-6)
            neg1 = cpool.tile([128, 1], f32)
            nc.vector.memset(neg1[:], -1.0)

            def chain(dxa, dxb, dya, dyb, P, width, px_tag):
                """dx = dxa - dxb etc; returns (px, py) fp16 [P, width]."""
                dx = tpool.tile([P, width], f16, tag="dx")
                dy = tpool.tile([P, width], f16, tag="dy")
                nc.vector.tensor_sub(dx[:], dxa, dxb)
                nc.vector.tensor_sub(dy[:], dya, dyb)
                sqx = tpool.tile([P, width], f16, tag="sqx")
                nc.scalar.activation(sqx[:], dx[:], AF.Square)
                sqy = tpool.tile([P, width], f16, tag="sqy")
                nc.scalar.activation(sqy[:], dy[:], AF.Square)
                nc.vector.tensor_add(sqx[:], sqx[:], sqy[:])
                rec = tpool.tile([P, width], f16, tag="rec")
                nc.scalar.activation(rec[:], sqx[:], AF.Abs_reciprocal_sqrt,
                                     bias=floor_b[:P])
                nc.scalar.activation(rec[:], rec[:], AF.Relu,
                                     bias=neg1[:P], scale=2.0 * SCALE * d_val)
                px = ppool.tile([P, width], f16, tag=f"px{px_tag}")
                py = ppool.tile([P, width], f16, tag=f"py{px_tag}")
                nc.vector.tensor_mul(px[:], rec[:], dx[:])
                nc.vector.tensor_mul(py[:], rec[:], dy[:])
                return px, py

            # ---- seam chain: rows {128,129,256,257}+R0 for all 12 offsets
            sax = spool.tile([SP, W], f32, tag="sa")
            nc.sync.dma_start(sax[:], sax_ext[:])
            sbx = spool.tile([SP, W], f32, tag="sb")
            nc.sync.dma_start(sbx[:], sbx_ext[:])
            say = spool.tile([SP, W], f32, tag="sa")
            nc.sync.dma_start(say[:], say_ext[:])
            sby = spool.tile([SP, W], f32, tag="sb")
            nc.sync.dma_start(sby[:], sby_ext[:])
            dxm = tpool.tile([SP, W], f16, tag="dx")
            dym = tpool.tile([SP, W], f16, tag="dy")
            nc.vector.tensor_sub(dxm[:], sax[:], sbx[:])
            nc.vector.tensor_sub(dym[:], say[:], sby[:])
            sqxm = tpool.tile([SP, W], f16, tag="sqx")
            nc.scalar.activation(sqxm[:], dxm[:], AF.Square)
            sqym = tpool.tile([SP, W], f16, tag="sqy")
            nc.scalar.activation(sqym[:], dym[:], AF.Square)
            nc.vector.tensor_add(sqxm[:], sqxm[:], sqym[:])
            recm_ = tpool.tile([SP, W], f16, tag="rec")
            nc.scalar.activation(recm_[:], sqxm[:], AF.Abs_reciprocal_sqrt,
                                 bias=floor_b[:SP])
            nc.scalar.activation(recm_[:], recm_[:], AF.Relu,
                                 bias=neg1[:SP], scale=2.0 * SCALE * d_val)
            pxm = cpool.tile([SP, W], f16, tag="pxm")
            pym = cpool.tile([SP, W], f16, tag="pym")
            nc.vector.tensor_mul(pxm[:], recm_[:], dxm[:])
            nc.vector.tensor_mul(pym[:], recm_[:], dym[:])

            for t in range(NT):
                t0 = t * TILE
                X = {}
                Y = {}
                for s in (0, -1, -2):
                    xt = xpool.tile([TILE, WS], f32, tag=f"xs{s}")
                    nc.sync.dma_start(
                        xt[:], xs_ext[t0 + s + 2: t0 + s + 2 + TILE, :])
                    X[s] = xt
                    yt = xpool.tile([TILE, WS], f32, tag=f"ys{s}")
                    nc.sync.dma_start(
                        yt[:], ys_ext[t0 + s + 2: t0 + s + 2 + TILE, :])
                    Y[s] = yt
                mask_t = xpool.tile([TILE, NX], f32, tag="mask")
                nc.sync.dma_start(mask_t[:], ms_ext[t0: t0 + TILE, :])

                fx_ps = psum_pool.tile([TILE, NX], f32, tag="fx")
                fy_ps = psum_pool.tile([TILE, NX], f32, tag="fy")

                for oi, (oy, ox) in enumerate(HOFF):
                    first = oi == 0
                    last = oi == NH - 1
                    # dx_o[p,u] = x[t0+p, u-2] - x[t0+p-oy, u-2-ox], u in [0,W)
                    px, py = chain(
                        X[0][:, 2: 2 + W], X[-oy][:, 2 - ox: 2 - ox + W],
                        Y[0][:, 2: 2 + W], Y[-oy][:, 2 - ox: 2 - ox + W],
                        TILE, W, "t")
                    if oi == 1:
                        # seam boundary mms (need only pxm/pym; must follow
                        # the bank-clearing start mms of oi==0)
                        for bi, bx in enumerate((-2, -1, 0, 1, 2)):
                            for pm, acc in ((pxm, fx_ps), (pym, fy_ps)):
                                for k in range(NX // 512):
                                    nc.tensor.matmul(
                                        acc[:, 512 * k: 512 * k + 512],
                                        lhsb_t[:, 128 * (2 * bi + t):
                                               128 * (2 * bi + t) + 128],
                                        pm[:, 2 + 512 * k + bx:
                                           2 + 512 * k + bx + 512],
                                        start=False, stop=False)
                    for ps, pm, acc in ((px, pxm, fx_ps), (py, pym, fy_ps)):
                        if ox == 0:
                            # merged lhsT block (I - S(oy)): one mm per chunk
                            for k in range(NX // 512):
                                nc.tensor.matmul(
                                    acc[:, 512 * k: 512 * k + 512],
                                    lhs_t[:, 128 * (3 + oy): 128 * (4 + oy)],
                                    ps[:, 2 + 512 * k: 2 + 512 * k + 512],
                                    start=first, stop=False)
                        else:
                            for k in range(NX // 512):
                                nc.tensor.matmul(
                                    acc[:, 512 * k: 512 * k + 512],
                                    lhs_t[:, 0:128],
                                    ps[:, 2 + 512 * k: 2 + 512 * k + 512],
                                    start=first, stop=False)
                            for k in range(NX // 512):
                                nc.tensor.matmul(
                                    acc[:, 512 * k: 512 * k + 512],
                                    lhs_t[:, 128 * (1 + oy): 128 * (2 + oy)],
                                    ps[:, 2 + 512 * k + ox:
                                       2 + 512 * k + ox + 512],
                                    start=False, stop=last)

                fx_sb = opool.tile([TILE, NX], f32, tag="fxsb")
                fy_sb = opool.tile([TILE, NX], f32, tag="fysb")
                for h in range(2):
                    hs = slice(1024 * h, 1024 * h + 1024)
                    nc.vector.scalar_tensor_tensor(
                        fx_sb[:, hs], fx_ps[:, hs], -float(kn_val) / SCALE,
                        mask_t[:, hs], OP.mult, OP.mult)
                    nc.sync.dma_start(out_ext[0, t0: t0 + TILE, hs],
                                      fx_sb[:, hs])
                    nc.vector.scalar_tensor_tensor(
                        fy_sb[:, hs], fy_ps[:, hs], -float(kn_val) / SCALE,
                        mask_t[:, hs], OP.mult, OP.mult)
                    nc.sync.dma_start(out_ext[1, t0: t0 + TILE, hs],
                                      fy_sb[:, hs])

    nc.compile()
    return nc


def _host_inputs(gx, gy, ms):
    """Per-core input dict list. gx/gy already scaled by 64 (f32-exact)."""
    eye = np.eye(128, dtype=np.float16)
    lhs = np.zeros((128, 6 * 128), dtype=np.float16)
    lhs[:, 0:128] = eye
    for oy in (0, 1, 2):
        blk = np.zeros((128, 128), dtype=np.float16)
        for m in range(128 - oy):
            blk[m + oy, m] = -1.0
        lhs[:, 128 * (1 + oy): 128 * (2 + oy)] = blk
        if oy > 0:
            lhs[:, 128 * (3 + oy): 128 * (4 + oy)] = eye + blk
    lhsb = np.zeros((48, 10 * 128), dtype=np.float16)
    for bi, bx in enumerate((-2, -1, 0, 1, 2)):
        for t in (0, 1):
            blk = np.zeros((48, 128), dtype=np.float16)
            for oy in (1, 2):
                oi = HOFF.index((oy, bx))
                for m in range(128 - oy, 128):
                    blk[4 * oi + 2 * t + (m + oy - 128), m] = -1.0
            lhsb[:, 128 * (2 * bi + t): 128 * (2 * bi + t) + 128] = blk

    cols = np.arange(-2, NX + 2) % NX         # width W, col u -> u-2
    colss = np.arange(-4, NX + 4) % NX        # width WS, col v -> v-4
    in_maps = []
    for i in range(NCORES):
        r0 = i * ROWS
        rows = np.arange(r0 - 2, r0 + ROWS + 2) % NY
        # seam rows per (oi, j): j in {0,1}: r0+128+j ; j in {2,3}: r0+256+(j-2)
        sa_rows = np.empty(4 * NH, dtype=np.int64)
        sb_rows = np.empty(4 * NH, dtype=np.int64)
        sb_cols = np.empty((4 * NH, W), dtype=np.int64)
        for oi, (oy, ox) in enumerate(HOFF):
            for j in range(4):
                row = r0 + 128 + j if j < 2 else r0 + 256 + (j - 2)
                sa_rows[4 * oi + j] = row % NY
                sb_rows[4 * oi + j] = (row - oy) % NY
                sb_cols[4 * oi + j] = (cols - ox) % NX
        in_maps.append({
            "xs": np.ascontiguousarray(gx[np.ix_(rows, colss)]),
            "ys": np.ascontiguousarray(gy[np.ix_(rows, colss)]),
            "ms": np.ascontiguousarray(ms[r0: r0 + ROWS, :]),
            "lhs": lhs,
            "lhsb": lhsb,
            "sax": np.ascontiguousarray(gx[sa_rows][:, cols]),
            "sbx": np.ascontiguousarray(gx[sb_rows[:, None], sb_cols]),
            "say": np.ascontiguousarray(gy[sa_rows][:, cols]),
            "sby": np.ascontiguousarray(gy[sb_rows[:, None], sb_cols]),
        })
    return in_maps


def _install_profile_hook():
    """The image's antenv lacks axon_hooks; recreate it so trace=True can
    drive NTFF profiling through libaxon_pjrt (local-only, no upload)."""
    import sys
    import types

    if "antenv.axon_hooks" not in sys.modules:
        mod = types.ModuleType("antenv.axon_hooks")
        holder = {}
        mod.set_axon_ntff_profile_hook = lambda h: holder.__setitem__("h", h)
        mod.get_axon_ntff_profile_hook = lambda: holder.get("h")
        sys.modules["antenv.axon_hooks"] = mod
        try:
            import antenv
            antenv.axon_hooks = mod
        except ImportError:
            pass
        if "/root/.axon_site" not in sys.path:
            sys.path.insert(0, "/root/.axon_site")
        from trn_agent_boot.trn_boot import _ntff_profile_via_ctypes
        h = _ntff_profile_via_ctypes("/opt/axon/libaxon_pjrt.so")
        if h is not None:
            mod.set_axon_ntff_profile_hook(h)
    from concourse import bass_utils as bu
    bu.upload_artifacts = lambda tmpdir: ""


def kernel(grid_x, grid_y, mask, d=1, kn=100, **_unused):
    global LAST_RESULTS
    from concourse.bass_utils import run_bass_kernel_spmd
    from concourse.bass_utils import checkenv

    if checkenv("KERNEL_TRACE"):
        _install_profile_hook()

    d_val = float(np.asarray(d))
    kn_val = float(np.asarray(kn))
    key = (d_val, kn_val)
    if key not in _cache:
        _cache[key] = _build(d_val, kn_val)
    nc = _cache[key]

    gx = np.asarray(grid_x, dtype=np.float32)[0, 0] * np.float32(SCALE)
    gy = np.asarray(grid_y, dtype=np.float32)[0, 0] * np.float32(SCALE)
    ms = np.asarray(mask, dtype=np.float32)[0, 0]
    in_maps = _host_inputs(gx, gy, ms)

    res = run_bass_kernel_spmd(nc, in_maps, core_ids=list(range(NCORES)),
                               trace=bool(checkenv("KERNEL_TRACE")))
    LAST_RESULTS = res

    fx = np.concatenate([res.results[i]["out"][0] for i in range(NCORES)], axis=0)
    fy = np.concatenate([res.results[i]["out"][1] for i in range(NCORES)], axis=0)
    fx = fx.reshape(1, 1, NY, NX)
    fy = fy.reshape(1, 1, NY, NX)
    return fx, fy
